# revision 10
# baseline (speedup 1.0000x reference)
"""Multi-head dot-product attention with prefix KV, on 8 trn2 NeuronCores.

Sharding: batch (2) x head-groups (4 groups of 4 heads) = 8 cores.
Each core computes q/k/v projections for its 4 heads, flash-style
attention (scores kept transposed: [kv, L] so no on-device transposes
are needed), and a partial out-projection [E, L]; the host sums the 4
head-group partials per batch and transposes back.

Key design points (v4):
  - bf16 everywhere (I/O + SBUF tiles); PSUM f32.  PE rate is 1
    cycle/row for bf16; DMA/SBUF cost halves vs f32.
  - all DRAM inputs are HOST-PRE-ARRANGED so every DMA lands as 128
    contiguous per-partition segments (a rearranging load costs ~1k
    tiny DMA descriptors and descriptor generation is the hidden
    serial resource; v2 spent ~250us there).
  - software-pipelined schedule: projection / out-projection matmuls
    are queued as filler units and pumped into the PE stream inside
    the attention chunk loop, so the PE never waits on softmax (ACT).
  - ctx matmuls are emitted one batch behind scores/exp of the same
    heads.
  - causal trimming: diagonal kv-chunks only compute scores/exp/ctx
    for the q columns they can see (widths 512/384/256/128); chunks
    are reordered [prefix, narrow diags, full chunks] so accumulation
    groups end on a full-width matmul.  Masking needs only a single
    128x128 triangle multiply per diagonal chunk.
  - softmax normalization is split per head-pair: the first pair's
    reciprocal/broadcast/multiply runs while the second pair's chunks
    stream, so only half the normalize chain sits on the tail.
  - denominators come free as ones-columns in V; partition broadcast
    of the reciprocals via K=1 matmuls.
"""

import numpy as np

B, LQ, LKV, E, H, D, P = 2, 2048, 2048, 1024, 16, 64, 64
NCORES = 8
HGROUPS = 4          # head groups (cores per batch)
HPC = H // HGROUPS   # heads per core = 4
KVPAD = 128 + LKV    # 2176
NCH = KVPAD // 128   # 17 chunks
NG = LQ // 512       # 4 L-groups of 512

_CACHE = {}


def _build_module(plan):
    """Build the single-core Bass module (same program for all 8 cores)."""
    import concourse.bass as bass
    import concourse.tile as tile
    import concourse.mybir as mybir
    from concourse import bacc
    from contextlib import ExitStack
    from collections import deque

    f32 = mybir.dt.float32
    f32r = mybir.dt.float32r
    bf16 = mybir.dt.bfloat16
    Exp = mybir.ActivationFunctionType.Exp

    chunks, info, nuniq = plan["chunks"], plan["info"], plan["nuniq"]

    nc = bacc.Bacc("TRN2", target_bir_lowering=False, debug=False,
                   enable_asserts=False, num_devices=NCORES)

    xqT_d = nc.dram_tensor("xqT", [NG, 128, 8, 512], bf16,
                           kind="ExternalInput").ap()
    xkv0_d = nc.dram_tensor("xkv0", [2, 128, 8, 256], bf16,
                            kind="ExternalInput").ap()
    xq0_d = nc.dram_tensor("xq0", [2, 128, 8, 256], bf16,
                           kind="ExternalInput").ap()
    xkvT_d = nc.dram_tensor("xkvT", [NG, 128, 8, 512], bf16,
                            kind="ExternalInput").ap()
    wq_d = nc.dram_tensor("wq", [128, 8, 256], bf16, kind="ExternalInput").ap()
    wk_d = nc.dram_tensor("wk", [128, 8, 256], bf16, kind="ExternalInput").ap()
    wv_d = nc.dram_tensor("wv", [128, 8, 256], bf16, kind="ExternalInput").ap()
    wo_d = nc.dram_tensor("wo", [128, 2, 1024], bf16, kind="ExternalInput").ap()
    kprefT_d = nc.dram_tensor("kprefT", [2, 128, 128], bf16,
                              kind="ExternalInput").ap()
    vpref_d = nc.dram_tensor("vpref", [128, HPC, 65], bf16,
                             kind="ExternalInput").ap()
    onescol_d = nc.dram_tensor("onescol", [4, 64], f32r,
                               kind="ExternalInput").ap()
    if nuniq:
        maskblk_d = nc.dram_tensor("maskblk", [nuniq, 128, 128], bf16,
                                   kind="ExternalInput").ap()
    outT_d = nc.dram_tensor("outT", [NG, 128, 8, 512], bf16,
                            kind="ExternalOutput").ap()

    with tile.TileContext(nc) as tc, ExitStack() as stk:
        pers = stk.enter_context(tc.tile_pool(name="pers", bufs=1))

        def ptile(shape, name, dt=bf16):
            return pers.tile(shape, dt, tag=name, name=name)

        wq_sb = ptile([128, 8, 256], "wq_sb")
        wk_sb = ptile([128, 8, 256], "wk_sb")
        wv_sb = ptile([128, 8, 256], "wv_sb")
        wo_sb = ptile([128, 2, 1024], "wo_sb")
        # QTS[hc][g]: q^T [2 heads x 64d, 512] per 512-L group.
        # KTS[hc][S]: S=0 prefix [128,128]; S=1..4 [128,512] (chunks 4S-3..4S).
        # VTS[c]: [128 kv, 4 heads, 65]: v at 0:64, ones column at 64.
        QTS = [[ptile([128, 512], f"QT{i}g{g}") for g in range(NG)]
               for i in range(2)]
        KTS = [[ptile([128, 128] if s == 0 else [128, 512], f"KT{i}s{s}")
                for s in range(5)] for i in range(2)]
        VTS = [ptile([128, HPC, 65], f"VT{c}") for c in range(NCH)]
        CTXT = [[ptile([128, 512], f"CTXT{i}g{g}") for g in range(NG)]
                for i in range(2)]
        ones_col = ptile([128, 64], "ones_col", f32r)

        def kslice(hc, c):
            if c == 0:
                return KTS[hc][0][:, 0:128]
            s, off = (c - 1) // 4 + 1, 128 * ((c - 1) % 4)
            return KTS[hc][s][:, off:off + 128]

        xio = stk.enter_context(tc.tile_pool(name="xio", bufs=2))
        attps = stk.enter_context(tc.tile_pool(name="att_ps", bufs=1, space="PSUM"))
        pjps = stk.enter_context(tc.tile_pool(name="pj_ps", bufs=1, space="PSUM"))
        attsb = stk.enter_context(tc.tile_pool(name="att_sb", bufs=1))

        # ---- x staging ----
        xq_t, xkv_t = {}, {}
        xq0h, xkv0h = [None, None], [None, None]

        def xload0():
            for hf in range(2):
                xkv0h[hf] = xio.tile([128, 8, 256], bf16, tag="xkv0", bufs=2,
                                     name="xkv0h")
                nc.sync.dma_start(out=xkv0h[hf], in_=xkv0_d[hf])
            nc.sync.dma_start(out=wk_sb, in_=wk_d)
            nc.sync.dma_start(out=wv_sb, in_=wv_d)
            for hf in range(2):
                xq0h[hf] = xio.tile([128, 8, 256], bf16, tag="xq0", bufs=2,
                                    name="xq0h")
                nc.sync.dma_start(out=xq0h[hf], in_=xq0_d[hf])
            nc.sync.dma_start(out=wq_sb, in_=wq_d)

        def xload(S):
            xkv_t[S] = xio.tile([128, 8, 512], bf16, tag="xkv", bufs=2,
                                name="xkv_t")
            nc.sync.dma_start(out=xkv_t[S], in_=xkvT_d[S])
            xq_t[S] = xio.tile([128, 8, 512], bf16, tag="xq", bufs=2, name="xq_t")
            nc.sync.dma_start(out=xq_t[S], in_=xqT_d[S])

        # ---- filler units: projections + out-projections ----
        def Ku0(t):
            ps = pjps.tile([128, 512], f32, tag="pj", bufs=2, name="ps_k")
            for hf in range(2):
                for ec in range(8):
                    nc.tensor.matmul(
                        ps[:, 256 * hf:256 * hf + 256],
                        lhsT=wk_sb[:, ec, 128 * t:128 * t + 128],
                        rhs=xkv0h[hf][:, ec, :], start=(ec == 0), stop=(ec == 7))
            nc.vector.tensor_copy(out=KTS[t][1], in_=ps)

        def Qu0(t):
            ps = pjps.tile([128, 512], f32, tag="pj", bufs=2, name="ps_q")
            for hf in range(2):
                for ec in range(8):
                    nc.tensor.matmul(
                        ps[:, 256 * hf:256 * hf + 256],
                        lhsT=wq_sb[:, ec, 128 * t:128 * t + 128],
                        rhs=xq0h[hf][:, ec, :], start=(ec == 0), stop=(ec == 7))
            nc.vector.tensor_copy(out=QTS[t][0], in_=ps)

        def Vu0(p):
            ps = pjps.tile([128, 512], f32, tag="pj", bufs=2, name="ps_v")
            for sub in range(2):
                for ec in range(8):
                    nc.tensor.matmul(
                        ps[:, 256 * sub:256 * sub + 256],
                        lhsT=xkv0h[p][:, ec, 128 * sub:128 * sub + 128],
                        rhs=wv_sb[:, ec, :], start=(ec == 0), stop=(ec == 7))
            for sub in range(2):
                c = 2 * p + sub + 1
                nc.vector.tensor_copy(
                    out=VTS[c][:, :, 0:D],
                    in_=ps[:, 256 * sub:256 * sub + 256].rearrange(
                        "p (h d) -> p h d", h=HPC))

        def Ku(S, t):
            ps = pjps.tile([128, 512], f32, tag="pj", bufs=2, name="ps_k")
            for ec in range(8):
                nc.tensor.matmul(ps, lhsT=wk_sb[:, ec, 128 * t:128 * t + 128],
                                 rhs=xkv_t[S][:, ec, :],
                                 start=(ec == 0), stop=(ec == 7))
            nc.vector.tensor_copy(out=KTS[t][S + 1], in_=ps)

        def Qu(S, t):
            ps = pjps.tile([128, 512], f32, tag="pj", bufs=2, name="ps_q")
            for ec in range(8):
                nc.tensor.matmul(ps, lhsT=wq_sb[:, ec, 128 * t:128 * t + 128],
                                 rhs=xq_t[S][:, ec, :],
                                 start=(ec == 0), stop=(ec == 7))
            nc.vector.tensor_copy(out=QTS[t][S], in_=ps)

        def Vu(S, p):
            ps = pjps.tile([128, 512], f32, tag="pj", bufs=2, name="ps_v")
            for sub in range(2):
                l0 = 128 * (2 * p + sub)
                for ec in range(8):
                    nc.tensor.matmul(
                        ps[:, 256 * sub:256 * sub + 256],
                        lhsT=xkv_t[S][:, ec, l0:l0 + 128],
                        rhs=wv_sb[:, ec, :], start=(ec == 0), stop=(ec == 7))
            for sub in range(2):
                c = 4 * S + 2 * p + sub + 1
                nc.vector.tensor_copy(
                    out=VTS[c][:, :, 0:D],
                    in_=ps[:, 256 * sub:256 * sub + 256].rearrange(
                        "p (h d) -> p h d", h=HPC))

        otg = {}

        def outproj_unit(g, et):
            ops = pjps.tile([128, 512], f32, tag="pj", bufs=2, name="ops")
            for hc in range(2):
                nc.tensor.matmul(ops, lhsT=wo_sb[:, hc, 128 * et:128 * et + 128],
                                 rhs=CTXT[hc][g], start=(hc == 0), stop=(hc == 1))
            if et == 0:
                otg[g] = attsb.tile([128, 8, 512], bf16, tag="otg", bufs=2,
                                    name="otg")
            if g == NG - 1:
                nc.scalar.copy(otg[g][:, et, :], ops)
            else:
                nc.vector.tensor_copy(out=otg[g][:, et, :], in_=ops)
            if et == 3:
                nc.sync.dma_start(out=outT_d[g][:, 0:4, :], in_=otg[g][:, 0:4, :])
            elif et == 7:
                nc.sync.dma_start(out=outT_d[g][:, 4:8, :], in_=otg[g][:, 4:8, :])

        # filler queue: (cost_ns, gate_group, fn)
        units = deque()
        debt = [0.0]

        def pump(ns):
            debt[0] += ns
            while units and debt[0] >= 0.6 * units[0][0]:
                cost, _, fn = units.popleft()
                fn()
                debt[0] -= cost

        def drain(need_g):
            while units and units[0][1] <= need_g:
                _, _, fn = units.popleft()
                fn()
            debt[0] = 0.0

        def supply_slice(S):
            if S == 0:
                for t in range(2):
                    units.append((1700, 0, (lambda t=t: Ku0(t))))
                for p in range(2):
                    units.append((1700, 0, (lambda p=p: Vu0(p))))
                for t in range(2):
                    units.append((1700, 0, (lambda t=t: Qu0(t))))
            else:
                for t in range(2):
                    units.append((1700, S, (lambda S=S, t=t: Ku(S, t))))
                for p in range(2):
                    units.append((1700, S, (lambda S=S, p=p: Vu(S, p))))
                for t in range(2):
                    units.append((1700, S, (lambda S=S, t=t: Qu(S, t))))

        # ---- attention ----
        def make_batches(g):
            """Exp batches of 1-2 chunks.  No scores-PSUM region may cross a
            2KB bank boundary: legal pairs are (512, w) and narrow pairs with
            w1 + w2 <= 512.  Chunk 0 opens (its start=True covers the full
            width); the last batch is a full-width chunk (clean stop)."""
            cs = chunks[g]
            W = {c: 512 - 128 * info[(g, c)][0] for c in cs}
            narrows = sorted([c for c in cs if c != 0 and W[c] < 512],
                             key=lambda c: W[c])
            fulls = [c for c in cs if c != 0 and W[c] == 512]
            batches = [[0]]
            if narrows:
                batches[0].append(narrows.pop(len(narrows) // 2))
            i, j = 0, len(narrows) - 1
            while i < j:
                if W[narrows[i]] + W[narrows[j]] <= 512:
                    batches.append([narrows[j], narrows[i]])
                    i += 1
                    j -= 1
                else:
                    batches.append([narrows[j]])
                    j -= 1
            if 0 <= i == j:
                batches.append([narrows[i]])
            while fulls:
                batches.append([fulls.pop(0)] +
                               ([fulls.pop(0)] if len(fulls) else []))
            return batches

        def attn_group(g, mts, finish_prev):
            batches = make_batches(g)
            nb = len(batches)
            pend = None
            for hp in range(2):
                heads = (2 * hp, 2 * hp + 1)
                ctx_ps = {h: attps.tile([128, 512], f32, tag="ctx", bufs=2,
                                        name=f"ctx{h}") for h in heads}
                prs = {}

                def emit_ctx(bi):
                    batch = batches[bi]
                    offs, _ = batch_layout(batch)
                    for h in heads:
                        pr = prs.pop((h, bi))
                        for j, c in enumerate(batch):
                            qlo, _ = info[(g, c)]
                            nc.tensor.matmul(
                                ctx_ps[h][0:65, 128 * qlo:512],
                                lhsT=VTS[c][:, h, :],
                                rhs=pr[:, offs[j]:offs[j] + 512 - 128 * qlo],
                                start=(bi == 0 and j == 0),
                                stop=(bi == nb - 1 and j == len(batch) - 1))

                def batch_layout(batch):
                    offs, w = [], 0
                    for c in batch:
                        offs.append(w)
                        w += 512 - 128 * info[(g, c)][0]
                    return offs, w

                for bi, batch in enumerate(batches):
                    offs, w = batch_layout(batch)
                    sc = {}
                    for hi, h in enumerate(heads):
                        sct = attps.tile([128, 1024], f32, tag="sc", bufs=2,
                                         name=f"sc{h}")
                        for j, c in enumerate(batch):
                            qlo = info[(g, c)][0]
                            prow = 64 * hi
                            nc.tensor.matmul(
                                sct[:, offs[j]:offs[j] + 512 - 128 * qlo],
                                lhsT=kslice(hp, c)[prow:prow + 64, :],
                                rhs=QTS[hp][g][prow:prow + 64, 128 * qlo:512],
                                start=True, stop=True)
                        sc[h] = sct
                    for hi, h in enumerate(heads):
                        pr = attsb.tile([128, 1024], bf16, tag="pr", bufs=4,
                                        name=f"pr{h}")
                        nc.scalar.activation(pr[:, 0:w], sc[h][:, 0:w], Exp)
                        for j, c in enumerate(batch):
                            qlo, mixed = info[(g, c)]
                            for sub, pid in mixed:
                                o = offs[j] + 128 * (sub - qlo)
                                nc.vector.tensor_mul(
                                    pr[:, o:o + 128], pr[:, o:o + 128],
                                    mts[pid])
                        prs[(h, bi)] = pr
                    pump(500)
                    if bi == 1 and finish_prev is not None:
                        finish_prev()
                        finish_prev = None
                    if bi == 1 and pend is not None:
                        normalize_hp(g, 0, *pend)
                        pend = None
                    if bi > 0:
                        emit_ctx(bi - 1)
                emit_ctx(nb - 1)
                # stage ctx to SBUF (frees PSUM), gather denominator rows
                dn = attsb.tile([33, 512], f32, tag="den", bufs=3, name="dn")
                nc.vector.memset(dn, 1.0)
                ctxs = {}
                for hi, h in enumerate(heads):
                    ctxs[h] = attsb.tile([65, 512], f32, tag="ctxs", bufs=4,
                                         name=f"ctxs{h}")
                    nc.vector.tensor_copy(out=ctxs[h], in_=ctx_ps[h][0:65, :])
                    nc.sync.dma_start(out=dn[32 * hi:32 * hi + 1, :],
                                      in_=ctxs[h][64:65, :])
                if hp == 0:
                    pend = (ctxs, dn)
                    pump(1200)
                else:
                    return ctxs, dn

        def normalize_hp(g, hp, ctxs, dn):
            rc = attsb.tile([33, 512], f32r, tag="rc", bufs=3, name="rc")
            with nc.allow_low_precision(reason="recip rounded to fp32r"):
                nc.vector.reciprocal(out=rc, in_=dn)
            for hi, h in enumerate((2 * hp, 2 * hp + 1)):
                row = 32 * hi
                bc = pjps.tile([128, 512], f32, tag="pj", bufs=2, name="bc")
                nc.tensor.matmul(bc[0:64, :],
                                 lhsT=ones_col[row:row + 1, :],
                                 rhs=rc[row:row + 1, :],
                                 start=True, stop=True,
                                 tile_position=(row, 0))
                if h % 2 == 0:
                    nc.vector.tensor_mul(CTXT[h // 2][g][0:64, :],
                                         ctxs[h][0:64, :], bc[0:64, :])
                else:
                    st = attsb.tile([64, 512], bf16, tag="stage", bufs=2,
                                    name="st")
                    nc.vector.tensor_mul(st, ctxs[h][0:64, :], bc[0:64, :])
                    nc.sync.dma_start(out=CTXT[h // 2][g][64:128, :], in_=st)

        # ---- schedule ----
        xload0()
        xload(1)
        nc.sync.dma_start(out=wo_sb, in_=wo_d)
        for hc in range(2):
            nc.sync.dma_start(out=KTS[hc][0], in_=kprefT_d[hc])
        nc.sync.dma_start(out=VTS[0], in_=vpref_d)
        oc_dest = bass.AP(tensor=ones_col.tensor, offset=ones_col.offset,
                          ap=[[32 * ones_col.ap[0][0], 4], list(ones_col.ap[1])])
        nc.sync.dma_start(out=oc_dest, in_=onescol_d)
        for c in range(1, NCH):
            nc.vector.memset(VTS[c][:, :, 64:65], 1.0)

        supply_slice(0)
        drain(0)          # slice 0 emitted directly (needed by group 0)
        mts = {}
        for pid in range(nuniq):
            mts[pid] = attsb.tile([128, 128], bf16, tag="mask",
                                  bufs=max(nuniq, 1), name=f"mt{pid}")
            nc.sync.dma_start(out=mts[pid], in_=maskblk_d[pid])
        supply_slice(1)
        finish_prev = None
        for g in range(NG):
            if g + 2 < NG:
                xload(g + 2)
            ctxs1, dn1 = attn_group(g, mts, finish_prev)
            if g + 2 < NG:
                supply_slice(g + 2)
            if g + 1 < NG:
                drain(g + 1)   # next group's K/V/Q filler, ahead of normalize
            def fin(g=g, c=ctxs1, d=dn1):
                normalize_hp(g, 1, c, d)
                for et in range(8):
                    units.append((450, NG,
                                  (lambda g=g, et=et: outproj_unit(g, et))))
            finish_prev = fin
        finish_prev()
        while units:
            _, _, fn = units.popleft()
            fn()

    nc.compile()
    return nc


def _make_plan(mask):
    """Block plan from the actual mask (union over batches -> one SPMD plan).

    Per (group, chunk): qlo = number of leading all-masked 128-q subblocks
    (scores/exp/ctx are trimmed to columns [128*qlo, 512)); mixed = list of
    (sub, pattern_id) 128x128 partially-masked subblocks.  Patterns are
    deduplicated (a causal mask has a single triangle pattern).
    """
    m = np.asarray(mask[:, 0])                       # [B, LQ, LKV] bool
    blk = m.reshape(B, NG, 4, 128, LKV // 128, 128)  # [B,g,sub,128q,cb,128kv]
    sub_any = blk.any(axis=(0, 3, 5))                # [NG, 4, 16]
    sub_all = blk.all(axis=(3, 5)).all(axis=0)       # [NG, 4, 16]
    blk_or = blk.any(axis=0)                         # [NG,4,128,16,128]
    chunks, info = [], {}
    pat_ids, pats = {}, []
    for g in range(NG):
        cl = [0]
        info[(g, 0)] = (0, [])
        for c in range(1, NCH):
            cb = c - 1
            if not sub_any[g, :, cb].any():
                continue
            cl.append(c)
            valid = [s for s in range(4) if sub_any[g, s, cb]]
            qlo = valid[0] if valid == list(range(valid[0], 4)) else 0
            mixed = []
            for s in range(qlo, 4):
                if sub_any[g, s, cb] and not sub_all[g, s, cb]:
                    pat = np.ascontiguousarray(
                        blk_or[g, s, :, cb, :].T)     # [128 kv, 128 q]
                    key = pat.tobytes()
                    if key not in pat_ids:
                        pat_ids[key] = len(pats)
                        pats.append(pat)
                    mixed.append((s, pat_ids[key]))
            info[(g, c)] = (qlo, mixed)
        chunks.append(cl)
    return {"chunks": chunks, "info": info, "nuniq": len(pats), "pats": pats}


def _prep_core_inputs(inputs, plan):
    """Per-core input dicts (8 cores: batch-major, then head-group)."""
    import ml_dtypes
    bf16 = ml_dtypes.bfloat16

    inputs_q = np.ascontiguousarray(inputs["inputs_q"], dtype=np.float32)
    inputs_kv = np.ascontiguousarray(inputs["inputs_kv"], dtype=np.float32)
    key_prefix = np.asarray(inputs["key_prefix"], dtype=np.float32)
    value_prefix = np.asarray(inputs["value_prefix"], dtype=np.float32)
    Wq = np.asarray(inputs["Wq"], dtype=np.float32)
    Wk = np.asarray(inputs["Wk"], dtype=np.float32)
    Wv = np.asarray(inputs["Wv"], dtype=np.float32)
    Wo = np.asarray(inputs["Wo"], dtype=np.float32)

    def xblock(x):
        # [E, L] -> [NG, 128, 8, 512] with E = ec*128 + p
        return np.ascontiguousarray(
            x.reshape(8, 128, NG, 512).transpose(2, 1, 0, 3).astype(bf16))

    xT = [xblock(inputs_q[b].T) for b in range(B)]
    xkT = [xblock(inputs_kv[b].T) for b in range(B)]
    # slice-0 half tiles for fast startup
    x0q = [np.ascontiguousarray(
        xT[b][0].reshape(128, 8, 2, 256).transpose(2, 0, 1, 3)) for b in range(B)]
    x0kv = [np.ascontiguousarray(
        xkT[b][0].reshape(128, 8, 2, 256).transpose(2, 0, 1, 3)) for b in range(B)]

    maskblk = np.stack(plan["pats"]).astype(bf16) if plan["nuniq"] else None

    in_maps = []
    for core in range(NCORES):
        b, hg = core // HGROUPS, core % HGROUPS
        hs = slice(HPC * hg, HPC * (hg + 1))
        kpT = key_prefix[b, :, hs, :]                 # [P, 4, D]
        kpT = kpT.transpose(1, 2, 0).reshape(2, 128, P)  # [hc, (2h x D), P]
        kpT = np.concatenate(
            [kpT, np.zeros((2, 128, 128 - P), np.float32)], axis=2)
        # chunk-0 V with ones columns baked in; pad rows (64..127) all-zero
        vp = np.zeros((128, HPC, 65), np.float32)
        vpref_b = value_prefix[b, :, hs, :]           # [P=64, 4, D]
        for h in range(HPC):
            vp[:P, h, 0:64] = vpref_b[:, h, :]
            vp[:P, h, 64] = 1.0
        im = {
            "xqT": xT[b],
            "xkvT": xkT[b],
            "xq0": x0q[b],
            "xkv0": x0kv[b],
            "wq": np.ascontiguousarray(
                (Wq[:, hs, :] / np.sqrt(D)).reshape(E, HPC * D)
                .reshape(8, 128, 256).transpose(1, 0, 2).astype(bf16)),
            "wk": np.ascontiguousarray(
                Wk[:, hs, :].reshape(E, HPC * D)
                .reshape(8, 128, 256).transpose(1, 0, 2).astype(bf16)),
            "wv": np.ascontiguousarray(
                Wv[:, hs, :].reshape(E, HPC * D)
                .reshape(8, 128, 256).transpose(1, 0, 2).astype(bf16)),
            "wo": np.ascontiguousarray(
                Wo[hs].reshape(HPC * D, E)
                .reshape(2, 128, 1024).transpose(1, 0, 2).astype(bf16)),
            "kprefT": np.ascontiguousarray(kpT.astype(bf16)),
            "vpref": np.ascontiguousarray(vp.astype(bf16)),
            "onescol": np.ones((4, 64), np.float32),
        }
        if plan["nuniq"]:
            im["maskblk"] = maskblk
        in_maps.append(im)
    return in_maps


def kernel(**inputs) -> np.ndarray:
    from concourse import bass_utils

    plan = _make_plan(inputs["mask"])
    key = (tuple(tuple(c) for c in plan["chunks"]),
           tuple(sorted((k, v[0], tuple(v[1])) for k, v in plan["info"].items())))
    if key not in _CACHE:
        _CACHE[key] = _build_module(plan)
    nc = _CACHE[key]

    in_maps = _prep_core_inputs(inputs, plan)
    res = bass_utils.run_bass_kernel_spmd(nc, in_maps, core_ids=list(range(NCORES)))

    out = np.zeros((B, LQ, E), np.float32)
    for core in range(NCORES):
        b = core // HGROUPS
        r = res.results[core]["outT"].astype(np.float32)   # [NG,128,8,512]
        out[b] += r.transpose(2, 1, 0, 3).reshape(E, LQ).T
    return out


# revision 12
# speedup vs baseline: 1.1357x; 1.1357x over previous
"""Multi-head dot-product attention with prefix KV, on 8 trn2 NeuronCores.

Sharding: batch (2) x head-groups (4 groups of 4 heads) = 8 cores.
Each core computes q/k/v projections for its 4 heads, flash-style
attention (scores kept transposed: [kv, L] so no on-device transposes
are needed), and a partial out-projection [E, L]; the host sums the 4
head-group partials per batch and transposes back.

Key design points (v4):
  - bf16 everywhere (I/O + SBUF tiles); PSUM f32.  PE rate is 1
    cycle/row for bf16; DMA/SBUF cost halves vs f32.
  - all DRAM inputs are HOST-PRE-ARRANGED so every DMA lands as 128
    contiguous per-partition segments (a rearranging load costs ~1k
    tiny DMA descriptors and descriptor generation is the hidden
    serial resource; v2 spent ~250us there).
  - software-pipelined schedule: projection / out-projection matmuls
    are queued as filler units and pumped into the PE stream inside
    the attention chunk loop, so the PE never waits on softmax (ACT).
  - ctx matmuls are emitted one batch behind scores/exp of the same
    heads.
  - causal trimming: diagonal kv-chunks only compute scores/exp/ctx
    for the q columns they can see (widths 512/384/256/128); chunks
    are reordered [prefix, narrow diags, full chunks] so accumulation
    groups end on a full-width matmul.  Masking needs only a single
    128x128 triangle multiply per diagonal chunk.
  - softmax normalization is split per head-pair: the first pair's
    reciprocal/broadcast/multiply runs while the second pair's chunks
    stream, so only half the normalize chain sits on the tail.
  - denominators come free as ones-columns in V; partition broadcast
    of the reciprocals via K=1 matmuls.
"""

import numpy as np

B, LQ, LKV, E, H, D, P = 2, 2048, 2048, 1024, 16, 64, 64
NCORES = 8
HGROUPS = 4          # head groups (cores per batch)
HPC = H // HGROUPS   # heads per core = 4
KVPAD = 128 + LKV    # 2176
NCH = KVPAD // 128   # 17 chunks
NG = LQ // 512       # 4 L-groups of 512

_CACHE = {}


def _build_module(plan):
    """Build the single-core Bass module (same program for all 8 cores)."""
    import concourse.bass as bass
    import concourse.tile as tile
    import concourse.mybir as mybir
    from concourse import bacc
    from contextlib import ExitStack
    from collections import deque

    f32 = mybir.dt.float32
    f32r = mybir.dt.float32r
    bf16 = mybir.dt.bfloat16
    Exp = mybir.ActivationFunctionType.Exp

    chunks, info, nuniq = plan["chunks"], plan["info"], plan["nuniq"]

    nc = bacc.Bacc("TRN2", target_bir_lowering=False, debug=False,
                   enable_asserts=False, num_devices=NCORES)

    xqT_d = nc.dram_tensor("xqT", [NG, 128, 8, 512], bf16,
                           kind="ExternalInput").ap()
    xkv0_d = nc.dram_tensor("xkv0", [2, 128, 8, 256], bf16,
                            kind="ExternalInput").ap()
    xq0_d = nc.dram_tensor("xq0", [2, 128, 8, 256], bf16,
                           kind="ExternalInput").ap()
    xkvT_d = nc.dram_tensor("xkvT", [NG, 128, 8, 512], bf16,
                            kind="ExternalInput").ap()
    wq_d = nc.dram_tensor("wq", [128, 8, 256], bf16, kind="ExternalInput").ap()
    wk_d = nc.dram_tensor("wk", [128, 8, 256], bf16, kind="ExternalInput").ap()
    wv_d = nc.dram_tensor("wv", [128, 8, 256], bf16, kind="ExternalInput").ap()
    wo_d = nc.dram_tensor("wo", [128, 2, 1024], bf16, kind="ExternalInput").ap()
    kprefT_d = nc.dram_tensor("kprefT", [2, 128, 128], bf16,
                              kind="ExternalInput").ap()
    vpref_d = nc.dram_tensor("vpref", [128, HPC, 65], bf16,
                             kind="ExternalInput").ap()
    if nuniq:
        maskblk_d = nc.dram_tensor("maskblk", [nuniq, 128, 128], bf16,
                                   kind="ExternalInput").ap()
    onescol_d = nc.dram_tensor("onescol", [4, 64], f32r,
                               kind="ExternalInput").ap()
    outT_d = nc.dram_tensor("outT", [NG, 128, 8, 512], bf16,
                            kind="ExternalOutput").ap()

    with tile.TileContext(nc) as tc, ExitStack() as stk:
        pers = stk.enter_context(tc.tile_pool(name="pers", bufs=1))

        def ptile(shape, name, dt=bf16):
            return pers.tile(shape, dt, tag=name, name=name)

        wq_sb = ptile([128, 8, 256], "wq_sb")
        wk_sb = ptile([128, 8, 256], "wk_sb")
        wv_sb = ptile([128, 8, 256], "wv_sb")
        wo_sb = ptile([128, 2, 1024], "wo_sb")
        # QTS[hc][g]: q^T [2 heads x 64d, 512] per 512-L group.
        # KTS[hc][S]: S=0 prefix [128,128]; S=1..4 [128,512] (chunks 4S-3..4S).
        # VTS[c]: [128 kv, 4 heads, 65]: v at 0:64, ones column at 64.
        QTS = [[ptile([128, 512], f"QT{i}g{g}") for g in range(NG)]
               for i in range(2)]
        KTS = [[ptile([128, 128] if s == 0 else [128, 512], f"KT{i}s{s}")
                for s in range(5)] for i in range(2)]
        VTS = [ptile([128, HPC, 65], f"VT{c}") for c in range(NCH)]
        CTXT = [[ptile([128, 512], f"CTXT{i}g{g}") for g in range(NG)]
                for i in range(2)]
        ones_col = ptile([128, 64], "ones_col", f32r)
        def kslice(hc, c):
            if c == 0:
                return KTS[hc][0][:, 0:128]
            s, off = (c - 1) // 4 + 1, 128 * ((c - 1) % 4)
            return KTS[hc][s][:, off:off + 128]

        xio = stk.enter_context(tc.tile_pool(name="xio", bufs=2))
        attps = stk.enter_context(tc.tile_pool(name="att_ps", bufs=1, space="PSUM"))
        pjps = stk.enter_context(tc.tile_pool(name="pj_ps", bufs=1, space="PSUM"))
        attsb = stk.enter_context(tc.tile_pool(name="att_sb", bufs=1))

        # ---- x staging ----
        xq_t, xkv_t = {}, {}
        xq0h, xkv0h = [None, None], [None, None]

        def xload0():
            for hf in range(2):
                xkv0h[hf] = xio.tile([128, 8, 256], bf16, tag="xkv0", bufs=2,
                                     name="xkv0h")
                nc.sync.dma_start(out=xkv0h[hf], in_=xkv0_d[hf])
            nc.sync.dma_start(out=wk_sb, in_=wk_d)
            nc.sync.dma_start(out=wv_sb, in_=wv_d)
            for hf in range(2):
                xq0h[hf] = xio.tile([128, 8, 256], bf16, tag="xq0", bufs=2,
                                    name="xq0h")
                nc.sync.dma_start(out=xq0h[hf], in_=xq0_d[hf])
            nc.sync.dma_start(out=wq_sb, in_=wq_d)

        def xload(S):
            xkv_t[S] = xio.tile([128, 8, 512], bf16, tag="xkv", bufs=2,
                                name="xkv_t")
            nc.sync.dma_start(out=xkv_t[S], in_=xkvT_d[S])
            xq_t[S] = xio.tile([128, 8, 512], bf16, tag="xq", bufs=2, name="xq_t")
            nc.sync.dma_start(out=xq_t[S], in_=xqT_d[S])

        # ---- filler units: projections + out-projections ----
        def Ku0(t):
            ps = pjps.tile([128, 512], f32, tag="pj", bufs=2, name="ps_k")
            for hf in range(2):
                for ec in range(8):
                    nc.tensor.matmul(
                        ps[:, 256 * hf:256 * hf + 256],
                        lhsT=wk_sb[:, ec, 128 * t:128 * t + 128],
                        rhs=xkv0h[hf][:, ec, :], start=(ec == 0), stop=(ec == 7))
            nc.vector.tensor_copy(out=KTS[t][1], in_=ps)

        def Qu0(t):
            ps = pjps.tile([128, 512], f32, tag="pj", bufs=2, name="ps_q")
            for hf in range(2):
                for ec in range(8):
                    nc.tensor.matmul(
                        ps[:, 256 * hf:256 * hf + 256],
                        lhsT=wq_sb[:, ec, 128 * t:128 * t + 128],
                        rhs=xq0h[hf][:, ec, :], start=(ec == 0), stop=(ec == 7))
            nc.vector.tensor_copy(out=QTS[t][0], in_=ps)

        def Vu0(p):
            ps = pjps.tile([128, 512], f32, tag="pj", bufs=2, name="ps_v")
            for sub in range(2):
                for ec in range(8):
                    nc.tensor.matmul(
                        ps[:, 256 * sub:256 * sub + 256],
                        lhsT=xkv0h[p][:, ec, 128 * sub:128 * sub + 128],
                        rhs=wv_sb[:, ec, :], start=(ec == 0), stop=(ec == 7))
            for sub in range(2):
                c = 2 * p + sub + 1
                nc.vector.tensor_copy(
                    out=VTS[c][:, :, 0:D],
                    in_=ps[:, 256 * sub:256 * sub + 256].rearrange(
                        "p (h d) -> p h d", h=HPC))

        def Ku(S, t):
            ps = pjps.tile([128, 512], f32, tag="pj", bufs=2, name="ps_k")
            for ec in range(8):
                nc.tensor.matmul(ps, lhsT=wk_sb[:, ec, 128 * t:128 * t + 128],
                                 rhs=xkv_t[S][:, ec, :],
                                 start=(ec == 0), stop=(ec == 7))
            nc.vector.tensor_copy(out=KTS[t][S + 1], in_=ps)

        def Qu(S, t):
            ps = pjps.tile([128, 512], f32, tag="pj", bufs=2, name="ps_q")
            for ec in range(8):
                nc.tensor.matmul(ps, lhsT=wq_sb[:, ec, 128 * t:128 * t + 128],
                                 rhs=xq_t[S][:, ec, :],
                                 start=(ec == 0), stop=(ec == 7))
            nc.vector.tensor_copy(out=QTS[t][S], in_=ps)

        def Vu(S, p):
            ps = pjps.tile([128, 512], f32, tag="pj", bufs=2, name="ps_v")
            for sub in range(2):
                l0 = 128 * (2 * p + sub)
                for ec in range(8):
                    nc.tensor.matmul(
                        ps[:, 256 * sub:256 * sub + 256],
                        lhsT=xkv_t[S][:, ec, l0:l0 + 128],
                        rhs=wv_sb[:, ec, :], start=(ec == 0), stop=(ec == 7))
            for sub in range(2):
                c = 4 * S + 2 * p + sub + 1
                nc.vector.tensor_copy(
                    out=VTS[c][:, :, 0:D],
                    in_=ps[:, 256 * sub:256 * sub + 256].rearrange(
                        "p (h d) -> p h d", h=HPC))

        otg = {}

        def outproj_unit(g, et):
            ops = pjps.tile([128, 512], f32, tag="pj", bufs=2, name="ops")
            for hc in range(2):
                nc.tensor.matmul(ops, lhsT=wo_sb[:, hc, 128 * et:128 * et + 128],
                                 rhs=CTXT[hc][g], start=(hc == 0), stop=(hc == 1))
            if et == 0:
                otg[g] = attsb.tile([128, 8, 512], bf16, tag="otg", bufs=2,
                                    name="otg")
            if g == NG - 1:
                nc.scalar.copy(otg[g][:, et, :], ops)
            else:
                nc.vector.tensor_copy(out=otg[g][:, et, :], in_=ops)
            if et == 3:
                nc.sync.dma_start(out=outT_d[g][:, 0:4, :], in_=otg[g][:, 0:4, :])
            elif et == 7:
                nc.sync.dma_start(out=outT_d[g][:, 4:8, :], in_=otg[g][:, 4:8, :])

        # filler queue: (cost_ns, gate_group, fn)
        units = deque()
        debt = [0.0]

        def pump(ns):
            debt[0] += ns
            while units and debt[0] >= 0.6 * units[0][0]:
                cost, _, fn = units.popleft()
                fn()
                debt[0] -= cost

        def drain(need_g):
            while units and units[0][1] <= need_g:
                _, _, fn = units.popleft()
                fn()
            debt[0] = 0.0

        def supply_slice(S):
            if S == 0:
                for t in range(2):
                    units.append((1700, 0, (lambda t=t: Ku0(t))))
                for t in range(2):
                    units.append((1700, 0, (lambda t=t: Qu0(t))))
                for p in range(2):
                    units.append((1700, 0, (lambda p=p: Vu0(p))))
            else:
                for t in range(2):
                    units.append((1700, S, (lambda S=S, t=t: Ku(S, t))))
                for t in range(2):
                    units.append((1700, S, (lambda S=S, t=t: Qu(S, t))))
                for p in range(2):
                    units.append((1700, S, (lambda S=S, p=p: Vu(S, p))))

        # ---- attention ----
        def make_batches(g):
            """Exp batches of 1-2 chunks.  No scores-PSUM region may cross a
            2KB bank boundary: legal pairs are (512, w) and narrow pairs with
            w1 + w2 <= 512.  Chunk 0 opens (its start=True covers the full
            width); the last batch is a full-width chunk (clean stop)."""
            cs = chunks[g]
            W = {c: 512 - 128 * info[(g, c)][0] for c in cs}
            narrows = sorted([c for c in cs if c != 0 and W[c] < 512],
                             key=lambda c: W[c])
            fulls = [c for c in cs if c != 0 and W[c] == 512]
            batches = [[0]]
            if narrows:
                batches[0].append(narrows.pop(len(narrows) // 2))
            i, j = 0, len(narrows) - 1
            while i < j:
                if W[narrows[i]] + W[narrows[j]] <= 512:
                    batches.append([narrows[j], narrows[i]])
                    i += 1
                    j -= 1
                else:
                    batches.append([narrows[j]])
                    j -= 1
            if 0 <= i == j:
                batches.append([narrows[i]])
            while fulls:
                batches.append([fulls.pop(0)] +
                               ([fulls.pop(0)] if len(fulls) else []))
            return batches

        def attn_group(g, mts, finish_prev):
            batches = make_batches(g)
            nb = len(batches)
            pend = None
            for hp in range(2):
                heads = (2 * hp, 2 * hp + 1)
                ctx_ps = {h: attps.tile([128, 512], f32, tag="ctx", bufs=2,
                                        name=f"ctx{h}") for h in heads}
                prs = {}

                def emit_ctx(bi):
                    batch = batches[bi]
                    offs, _ = batch_layout(batch)
                    for h in heads:
                        pr = prs.pop((h, bi))
                        for j, c in enumerate(batch):
                            qlo, _ = info[(g, c)]
                            nc.tensor.matmul(
                                ctx_ps[h][0:65, 128 * qlo:512],
                                lhsT=VTS[c][:, h, :],
                                rhs=pr[:, offs[j]:offs[j] + 512 - 128 * qlo],
                                start=(bi == 0 and j == 0),
                                stop=(bi == nb - 1 and j == len(batch) - 1))

                def batch_layout(batch):
                    offs, w = [], 0
                    for c in batch:
                        offs.append(w)
                        w += 512 - 128 * info[(g, c)][0]
                    return offs, w

                for bi, batch in enumerate(batches):
                    offs, w = batch_layout(batch)
                    sc = {}
                    for hi, h in enumerate(heads):
                        sct = attps.tile([128, 1024], f32, tag="sc", bufs=2,
                                         name=f"sc{h}")
                        for j, c in enumerate(batch):
                            qlo = info[(g, c)][0]
                            prow = 64 * hi
                            nc.tensor.matmul(
                                sct[:, offs[j]:offs[j] + 512 - 128 * qlo],
                                lhsT=kslice(hp, c)[prow:prow + 64, :],
                                rhs=QTS[hp][g][prow:prow + 64, 128 * qlo:512],
                                start=True, stop=True)
                        sc[h] = sct
                    for hi, h in enumerate(heads):
                        pr = attsb.tile([128, 1024], bf16, tag="pr", bufs=4,
                                        name=f"pr{h}")
                        nc.scalar.activation(pr[:, 0:w], sc[h][:, 0:w], Exp)
                        for j, c in enumerate(batch):
                            qlo, mixed = info[(g, c)]
                            for sub, pid in mixed:
                                o = offs[j] + 128 * (sub - qlo)
                                nc.vector.tensor_mul(
                                    pr[:, o:o + 128], pr[:, o:o + 128],
                                    mts[pid])
                        prs[(h, bi)] = pr
                    pump(850 if (hp == 0 or bi < nb - 3) else 1600)
                    if bi == 1 and finish_prev is not None:
                        finish_prev()
                        finish_prev = None
                    if bi == 2 and pend is not None:
                        normalize_hp(g, 0, *pend)
                        pend = None
                    if bi > 0:
                        emit_ctx(bi - 1)
                emit_ctx(nb - 1)
                # stage ctx to SBUF (frees PSUM), gather denominator rows
                dn = attsb.tile([33, 512], f32, tag="den", bufs=3, name="dn")
                nc.vector.memset(dn, 1.0)
                ctxs = {}
                for hi, h in enumerate(heads):
                    ctxs[h] = attsb.tile([65, 512], f32, tag="ctxs", bufs=4,
                                         name=f"ctxs{h}")
                    if g == NG - 1 and hp == 1:
                        nc.scalar.copy(ctxs[h], ctx_ps[h][0:65, :])
                    else:
                        nc.vector.tensor_copy(out=ctxs[h], in_=ctx_ps[h][0:65, :])
                    nc.sync.dma_start(out=dn[32 * hi:32 * hi + 1, :],
                                      in_=ctxs[h][64:65, :])
                if hp == 0:
                    pend = (ctxs, dn)
                    pump(1200)
                else:
                    return ctxs, dn

        def normalize_hp(g, hp, ctxs, dn):
            rc = attsb.tile([33, 512], f32r, tag="rc", bufs=3, name="rc")
            with nc.allow_low_precision(reason="recip rounded to fp32r"):
                nc.vector.reciprocal(out=rc, in_=dn)
            for hi, h in enumerate((2 * hp, 2 * hp + 1)):
                row = 32 * hi
                bc = pjps.tile([128, 512], f32, tag="pj", bufs=2, name="bc")
                nc.tensor.matmul(bc[0:64, :],
                                 lhsT=ones_col[row:row + 1, :],
                                 rhs=rc[row:row + 1, :],
                                 start=True, stop=True,
                                 tile_position=(row, 0))
                if h % 2 == 0:
                    nc.vector.tensor_mul(CTXT[h // 2][g][0:64, :],
                                         ctxs[h][0:64, :], bc[0:64, :])
                else:
                    st = attsb.tile([64, 512], bf16, tag="stage", bufs=2,
                                    name="st")
                    nc.vector.tensor_mul(st, ctxs[h][0:64, :], bc[0:64, :])
                    nc.sync.dma_start(out=CTXT[h // 2][g][64:128, :], in_=st)

        # ---- schedule ----
        xload0()
        xload(1)
        nc.sync.dma_start(out=wo_sb, in_=wo_d)
        for hc in range(2):
            nc.sync.dma_start(out=KTS[hc][0], in_=kprefT_d[hc])
        nc.sync.dma_start(out=VTS[0], in_=vpref_d)
        oc_dest = bass.AP(tensor=ones_col.tensor, offset=ones_col.offset,
                          ap=[[32 * ones_col.ap[0][0], 4], list(ones_col.ap[1])])
        nc.sync.dma_start(out=oc_dest, in_=onescol_d)
        for c in range(1, NCH):
            nc.vector.memset(VTS[c][:, :, 64:65], 1.0)

        supply_slice(0)
        drain(0)          # slice 0 emitted directly (needed by group 0)
        mts = {}
        for pid in range(nuniq):
            mts[pid] = attsb.tile([128, 128], bf16, tag="mask",
                                  bufs=max(nuniq, 1), name=f"mt{pid}")
            nc.sync.dma_start(out=mts[pid], in_=maskblk_d[pid])
        supply_slice(1)
        finish_prev = None
        for g in range(NG):
            if g + 2 < NG:
                xload(g + 2)
            ctxs1, dn1 = attn_group(g, mts, finish_prev)
            if g + 2 < NG:
                supply_slice(g + 2)
            if g + 1 < NG:
                drain(g + 1)   # next group's K/V/Q filler, ahead of normalize
            def fin(g=g, c=ctxs1, d=dn1):
                normalize_hp(g, 1, c, d)
                for et in range(8):
                    units.append((450, NG,
                                  (lambda g=g, et=et: outproj_unit(g, et))))
            finish_prev = fin
        finish_prev()
        while units:
            _, _, fn = units.popleft()
            fn()

    nc.compile()
    return nc


def _make_plan(mask):
    """Block plan from the actual mask (union over batches -> one SPMD plan).

    Per (group, chunk): qlo = number of leading all-masked 128-q subblocks
    (scores/exp/ctx are trimmed to columns [128*qlo, 512)); mixed = list of
    (sub, pattern_id) 128x128 partially-masked subblocks.  Patterns are
    deduplicated (a causal mask has a single triangle pattern).
    """
    m = np.asarray(mask[:, 0])                       # [B, LQ, LKV] bool
    blk = m.reshape(B, NG, 4, 128, LKV // 128, 128)  # [B,g,sub,128q,cb,128kv]
    sub_any = blk.any(axis=(0, 3, 5))                # [NG, 4, 16]
    sub_all = blk.all(axis=(3, 5)).all(axis=0)       # [NG, 4, 16]
    blk_or = blk.any(axis=0)                         # [NG,4,128,16,128]
    chunks, info = [], {}
    pat_ids, pats = {}, []
    for g in range(NG):
        cl = [0]
        info[(g, 0)] = (0, [])
        for c in range(1, NCH):
            cb = c - 1
            if not sub_any[g, :, cb].any():
                continue
            cl.append(c)
            valid = [s for s in range(4) if sub_any[g, s, cb]]
            qlo = valid[0] if valid == list(range(valid[0], 4)) else 0
            mixed = []
            for s in range(qlo, 4):
                if sub_any[g, s, cb] and not sub_all[g, s, cb]:
                    pat = np.ascontiguousarray(
                        blk_or[g, s, :, cb, :].T)     # [128 kv, 128 q]
                    key = pat.tobytes()
                    if key not in pat_ids:
                        pat_ids[key] = len(pats)
                        pats.append(pat)
                    mixed.append((s, pat_ids[key]))
            info[(g, c)] = (qlo, mixed)
        chunks.append(cl)
    return {"chunks": chunks, "info": info, "nuniq": len(pats), "pats": pats}


def _prep_core_inputs(inputs, plan):
    """Per-core input dicts (8 cores: batch-major, then head-group)."""
    import ml_dtypes
    bf16 = ml_dtypes.bfloat16

    inputs_q = np.ascontiguousarray(inputs["inputs_q"], dtype=np.float32)
    inputs_kv = np.ascontiguousarray(inputs["inputs_kv"], dtype=np.float32)
    key_prefix = np.asarray(inputs["key_prefix"], dtype=np.float32)
    value_prefix = np.asarray(inputs["value_prefix"], dtype=np.float32)
    Wq = np.asarray(inputs["Wq"], dtype=np.float32)
    Wk = np.asarray(inputs["Wk"], dtype=np.float32)
    Wv = np.asarray(inputs["Wv"], dtype=np.float32)
    Wo = np.asarray(inputs["Wo"], dtype=np.float32)

    def xblock(x):
        # [E, L] -> [NG, 128, 8, 512] with E = ec*128 + p
        return np.ascontiguousarray(
            x.reshape(8, 128, NG, 512).transpose(2, 1, 0, 3).astype(bf16))

    xT = [xblock(inputs_q[b].T) for b in range(B)]
    xkT = [xblock(inputs_kv[b].T) for b in range(B)]
    # slice-0 half tiles for fast startup
    x0q = [np.ascontiguousarray(
        xT[b][0].reshape(128, 8, 2, 256).transpose(2, 0, 1, 3)) for b in range(B)]
    x0kv = [np.ascontiguousarray(
        xkT[b][0].reshape(128, 8, 2, 256).transpose(2, 0, 1, 3)) for b in range(B)]

    maskblk = np.stack(plan["pats"]).astype(bf16) if plan["nuniq"] else None

    in_maps = []
    for core in range(NCORES):
        b, hg = core // HGROUPS, core % HGROUPS
        hs = slice(HPC * hg, HPC * (hg + 1))
        kpT = key_prefix[b, :, hs, :]                 # [P, 4, D]
        kpT = kpT.transpose(1, 2, 0).reshape(2, 128, P)  # [hc, (2h x D), P]
        kpT = np.concatenate(
            [kpT, np.zeros((2, 128, 128 - P), np.float32)], axis=2)
        # chunk-0 V with ones columns baked in; pad rows (64..127) all-zero
        vp = np.zeros((128, HPC, 65), np.float32)
        vpref_b = value_prefix[b, :, hs, :]           # [P=64, 4, D]
        for h in range(HPC):
            vp[:P, h, 0:64] = vpref_b[:, h, :]
            vp[:P, h, 64] = 1.0
        im = {
            "xqT": xT[b],
            "xkvT": xkT[b],
            "xq0": x0q[b],
            "xkv0": x0kv[b],
            "wq": np.ascontiguousarray(
                (Wq[:, hs, :] / np.sqrt(D)).reshape(E, HPC * D)
                .reshape(8, 128, 256).transpose(1, 0, 2).astype(bf16)),
            "wk": np.ascontiguousarray(
                Wk[:, hs, :].reshape(E, HPC * D)
                .reshape(8, 128, 256).transpose(1, 0, 2).astype(bf16)),
            "wv": np.ascontiguousarray(
                Wv[:, hs, :].reshape(E, HPC * D)
                .reshape(8, 128, 256).transpose(1, 0, 2).astype(bf16)),
            "wo": np.ascontiguousarray(
                Wo[hs].reshape(HPC * D, E)
                .reshape(2, 128, 1024).transpose(1, 0, 2).astype(bf16)),
            "kprefT": np.ascontiguousarray(kpT.astype(bf16)),
            "vpref": np.ascontiguousarray(vp.astype(bf16)),
            "onescol": np.ones((4, 64), np.float32),
        }
        if plan["nuniq"]:
            im["maskblk"] = maskblk
        in_maps.append(im)
    return in_maps


def kernel(**inputs) -> np.ndarray:
    from concourse import bass_utils

    plan = _make_plan(inputs["mask"])
    key = (tuple(tuple(c) for c in plan["chunks"]),
           tuple(sorted((k, v[0], tuple(v[1])) for k, v in plan["info"].items())))
    if key not in _CACHE:
        _CACHE[key] = _build_module(plan)
    nc = _CACHE[key]

    in_maps = _prep_core_inputs(inputs, plan)
    res = bass_utils.run_bass_kernel_spmd(nc, in_maps, core_ids=list(range(NCORES)))

    out = np.zeros((B, LQ, E), np.float32)
    for core in range(NCORES):
        b = core // HGROUPS
        r = res.results[core]["outT"].astype(np.float32)   # [NG,128,8,512]
        out[b] += r.transpose(2, 1, 0, 3).reshape(E, LQ).T
    return out
